# revision 2
# baseline (speedup 1.0000x reference)
"""Laplace attention kernel for Trainium2, 8 NeuronCores.

Math (per batch b):
  k = MLP_k(x1[b])  [NK, D];  q = MLP_q(x2[b])  [NQ, D]
  dist[i,j] = sum_d |k[j,d] - q[i,d]|
  out = softmax_j(-dist) @ r[b]

Distribution: core c = (b, h) = (c//2, c%2): batch b, query-half h (256 queries).
Keys/values are replicated per batch pair of cores.

Per-core algorithm:
  - MLPs run transposed on the PE: kT2 [128=(i2,d), NK] holds kT stacked twice,
    q2T [128=(i2,d), 128] holds qT for queries (i2*128 + p).
  - For each query pair p, a [128, NK] tile M_p is produced:
      min-form pairs (DVE):  M_p = min(kT2, q_p)        (tensor_scalar, 2x fp32)
      abs-form pairs (ACT):  M_p = |kT2 - q_p|          (activation Abs, bias=q, scale=-1)
    Then one PE matmul per 512-column window reduces over the 128 partitions
    with a constant ones-block lhsT whose coefficient is -2 for min-form
    columns and +1 for abs-form columns, accumulating 32 pairs into one
    [64, 1024] PSUM tile. A K=1 correction matmul adds A_j = sum_d k[j,d]
    to min-form rows only (masked lhsT). The remaining B_i = sum_d q[i,d]
    offset is constant per row and cancels in softmax, and dist is large
    and positive, so exp(-P) needs no max-subtraction.
  - softmax: ACT Exp (scale=-1) with accum_out row-sum, DVE reciprocal +
    scale; weights stored bf16.
  - value matmul: DMA-transpose the bf16 weights to [j, q] layout, then PE
    per 128-key tile with r (bf16) as stationary operand, accumulating in
    PSUM; result is [D, queries], written out and transposed on the host.
"""

import os
import numpy as np
import ml_dtypes

import concourse.bass as bass
import concourse.mybir as mybir
from concourse.tile import TileContext
from concourse import bass_utils

B, NQ, NK, D = 4, 512, 1024, 64
NCORES = 8
QSH = NQ // 2           # queries per core
NPAIR = QSH // 2        # 128 query pairs per core
NWIN = NK // 512        # 512-column matmul windows
ACT_CNT_PER16 = 5       # s%16 < 5 -> abs-form pair on ACT engine

F32 = mybir.dt.float32
F32R = mybir.dt.float32r
BF16 = mybir.dt.bfloat16

LAST_RESULT = None      # BassKernelResults of the most recent run (for test.py)


def _is_act_pair(s):
    return (s % 16) < ACT_CNT_PER16


# ---------------------------------------------------------------------------
# walrus workaround: the CTRL-class instructions (Drain etc.) can carry only a
# few sem waits; hoist excess waits onto injected NoOps on the same engine.
def _split_excess_waits(nc, max_waits=1):
    for f in nc.m.functions:
        for bb in f.blocks:
            new_insts = []
            for inst in bb.instructions:
                si = inst.sync_info
                if si is not None and si.on_wait and len(si.on_wait) > max_waits:
                    waits = list(si.on_wait)
                    excess, keep = waits[:-max_waits], waits[-max_waits:]
                    for i in range(0, len(excess), max_waits):
                        nop = mybir.InstNoOp(
                            name=f"{inst.name}_waitsplit_{i // max_waits}",
                            ins=[], outs=[])
                        nop.engine = inst.engine
                        nop.sync_info = mybir.SyncInfo(
                            on_wait=excess[i:i + max_waits], on_update=[])
                        new_insts.append(nop)
                    si.on_wait = keep
                new_insts.append(inst)
            bb.instructions = new_insts


# shim antenv.axon_hooks (absent in this image) so BASS_TRACE=1 profiling works
def _install_ntff_shim():
    import sys, types
    if 'antenv.axon_hooks' in sys.modules:
        return
    try:
        mod = types.ModuleType('antenv.axon_hooks')
        state = {}
        mod.set_axon_ntff_profile_hook = lambda h: state.__setitem__('h', h)
        mod.get_axon_ntff_profile_hook = lambda: state.get('h')
        sys.modules['antenv.axon_hooks'] = mod
        import antenv
        antenv.axon_hooks = mod
        from trn_agent_boot.trn_boot import _ntff_profile_via_ctypes
        h = _ntff_profile_via_ctypes('/opt/axon/libaxon_pjrt.so')
        if h is not None:
            mod.set_axon_ntff_profile_hook(h)
    except Exception:
        pass


# ---------------------------------------------------------------------------
def _build_program():
    nc = bass.Bass("TRN2")

    x1t = nc.dram_tensor("x1t", [D, NK], F32, kind="ExternalInput")
    x2t = nc.dram_tensor("x2t", [D, QSH], F32, kind="ExternalInput")
    rv = nc.dram_tensor("rv", [NK, D], BF16, kind="ExternalInput")
    wk1 = nc.dram_tensor("wk1", [D, D], F32, kind="ExternalInput")
    bk1 = nc.dram_tensor("bk1", [D, 1], F32, kind="ExternalInput")
    wk2d = nc.dram_tensor("wk2d", [D, 128], F32, kind="ExternalInput")
    bk2d = nc.dram_tensor("bk2d", [128, 1], F32, kind="ExternalInput")
    wq1 = nc.dram_tensor("wq1", [D, D], F32, kind="ExternalInput")
    bq1 = nc.dram_tensor("bq1", [D, 1], F32, kind="ExternalInput")
    wq2 = nc.dram_tensor("wq2", [D, D], F32, kind="ExternalInput")
    bq2d = nc.dram_tensor("bq2d", [128, 1], F32, kind="ExternalInput")
    wones = nc.dram_tensor("wones", [128, 32 * 64], F32R, kind="ExternalInput")
    cmask = nc.dram_tensor("cmask", [1, 64], F32R, kind="ExternalInput")
    ones64 = nc.dram_tensor("ones64", [D, 1], F32, kind="ExternalInput")
    yout = nc.dram_tensor("yout", [2, D, 128], F32, kind="ExternalOutput")

    ACT = mybir.ActivationFunctionType
    ALU = mybir.AluOpType

    with TileContext(nc) as tc:
        import contextlib
        with contextlib.ExitStack() as ctx:
            consts = ctx.enter_context(tc.tile_pool(name="consts", bufs=1))

            x1t_sb = consts.tile([D, NK], F32)
            x2t_sb = consts.tile([D, QSH], F32)
            r_sb = consts.tile([128, 8 * D], BF16)
            wk1_sb = consts.tile([D, D], F32)
            bk1_sb = consts.tile([D, 1], F32)
            wk2d_sb = consts.tile([D, 128], F32)
            bk2d_sb = consts.tile([128, 1], F32)
            wq1_sb = consts.tile([D, D], F32)
            bq1_sb = consts.tile([D, 1], F32)
            wq2_sb = consts.tile([D, D], F32)
            bq2d_sb = consts.tile([128, 1], F32)
            wones_sb = consts.tile([128, 32 * 64], F32R)
            cmask_sb = consts.tile([1, 64], F32R)
            ones64_sb = consts.tile([D, 1], F32)

            nc.sync.dma_start(out=x1t_sb[:], in_=x1t[:, :])
            nc.sync.dma_start(out=x2t_sb[:], in_=x2t[:, :])
            for jt in range(8):
                nc.sync.dma_start(out=r_sb[:, jt * D:(jt + 1) * D],
                                  in_=rv[jt * 128:(jt + 1) * 128, :])
            nc.sync.dma_start(out=wk1_sb[:], in_=wk1[:, :])
            nc.sync.dma_start(out=bk1_sb[:], in_=bk1[:, :])
            nc.sync.dma_start(out=wk2d_sb[:], in_=wk2d[:, :])
            nc.sync.dma_start(out=bk2d_sb[:], in_=bk2d[:, :])
            nc.sync.dma_start(out=wq1_sb[:], in_=wq1[:, :])
            nc.sync.dma_start(out=bq1_sb[:], in_=bq1[:, :])
            nc.sync.dma_start(out=wq2_sb[:], in_=wq2[:, :])
            nc.sync.dma_start(out=bq2d_sb[:], in_=bq2d[:, :])
            nc.sync.dma_start(out=wones_sb[:], in_=wones[:, :])
            nc.sync.dma_start(out=cmask_sb[:], in_=cmask[:, :])
            nc.sync.dma_start(out=ones64_sb[:], in_=ones64[:, :])

            kt2_sb = consts.tile([128, NK], F32)
            q2t_sb = consts.tile([128, 128], F32)
            ht_sb = consts.tile([D, NK], F32)
            hqt_sb = consts.tile([D, QSH], F32)
            arow_sb = consts.tile([1, NK], F32R)

            # ---- MLPs (transposed) ----
            with tc.tile_pool(name="mlppsum", bufs=2, space="PSUM") as mp:
                for w in range(NWIN):
                    ph = mp.tile([D, 512], F32, tag="ph")
                    nc.tensor.matmul(ph[:], wk1_sb[:], x1t_sb[:, w * 512:(w + 1) * 512],
                                     start=True, stop=True)
                    nc.scalar.activation(ht_sb[:, w * 512:(w + 1) * 512], ph[:],
                                         ACT.Relu, bias=bk1_sb[:, 0:1], scale=1.0)
                for w in range(NWIN):
                    pk = mp.tile([128, 512], F32, tag="pk")
                    nc.tensor.matmul(pk[:], wk2d_sb[:], ht_sb[:, w * 512:(w + 1) * 512],
                                     start=True, stop=True)
                    nc.scalar.activation(kt2_sb[:, w * 512:(w + 1) * 512], pk[:],
                                         ACT.Identity, bias=bk2d_sb[:, 0:1], scale=1.0)
                phq = mp.tile([D, QSH], F32, tag="ph")
                nc.tensor.matmul(phq[:], wq1_sb[:], x2t_sb[:], start=True, stop=True)
                nc.scalar.activation(hqt_sb[:], phq[:], ACT.Relu,
                                     bias=bq1_sb[:, 0:1], scale=1.0)
                pq = mp.tile([128, 128], F32, tag="pk")
                nc.tensor.matmul(pq[0:64, :], wq2_sb[:], hqt_sb[:, 0:128],
                                 start=True, stop=False, skip_group_check=True)
                nc.tensor.matmul(pq[64:128, :], wq2_sb[:], hqt_sb[:, 128:256],
                                 start=True, stop=True, skip_group_check=True)
                nc.scalar.activation(q2t_sb[:], pq[:], ACT.Identity,
                                     bias=bq2d_sb[:, 0:1], scale=1.0)
                # A_j = sum_d k[j, d]  (fp32 exact), stored f32r for corrections
                pa = mp.tile([1, NK], F32, tag="pa")
                for w in range(NWIN):
                    nc.tensor.matmul(pa[:, w * 512:(w + 1) * 512], ones64_sb[:],
                                     kt2_sb[0:64, w * 512:(w + 1) * 512],
                                     start=True, stop=True, skip_group_check=True)
                nc.vector.tensor_copy(arow_sb[:], pa[:])

            # ---- main loop ----
            mpool = ctx.enter_context(tc.tile_pool(name="mtiles", bufs=3))
            dpool = ctx.enter_context(
                tc.tile_pool(name="dist", bufs=3, space="PSUM"))
            opool = ctx.enter_context(
                tc.tile_pool(name="outp", bufs=2, space="PSUM"))
            spool = ctx.enter_context(tc.tile_pool(name="smax", bufs=3))
            otpool = ctx.enter_context(tc.tile_pool(name="outs", bufs=2))

            for rr in range(2):
                out_g = []
                for g in range(2):
                    dist = dpool.tile([64, NK], F32, tag="dist")
                    for s in range(32):
                        p = rr * 64 + g * 32 + s
                        mt = mpool.tile([128, NK], F32R, tag="mt")
                        if _is_act_pair(s):
                            nc.scalar.activation(mt[:], kt2_sb[:], ACT.Abs,
                                                 bias=q2t_sb[:, p:p + 1], scale=-1.0)
                        else:
                            nc.vector.tensor_scalar(mt[:], kt2_sb[:],
                                                    q2t_sb[:, p:p + 1], None, ALU.min)
                        for w in range(NWIN):
                            nc.tensor.matmul(
                                dist[:, w * 512:(w + 1) * 512],
                                wones_sb[:, s * 64:(s + 1) * 64],
                                mt[:, w * 512:(w + 1) * 512],
                                start=(s == 0), stop=False, skip_group_check=True)
                    # A_j correction on min-form rows only (masked K=1 matmul)
                    for w in range(NWIN):
                        nc.tensor.matmul(
                            dist[:, w * 512:(w + 1) * 512],
                            cmask_sb[:],
                            arow_sb[:, w * 512:(w + 1) * 512],
                            start=False, stop=True, skip_group_check=True)

                    # ---- softmax over keys (no shift needed: dist >> 0) ----
                    expw = spool.tile([64, NK], BF16, tag="expw")
                    ssum = spool.tile([64, 1], F32, tag="ssum")
                    rcol = spool.tile([64, 1], F32, tag="rcol")
                    nc.scalar.activation(expw[:], dist[:], ACT.Exp,
                                         bias=0.0, scale=-1.0, accum_out=ssum[:])
                    nc.vector.reciprocal(rcol[:], ssum[:])
                    expn = spool.tile([64, NK], BF16, tag="expn")
                    nc.vector.tensor_scalar(expn[:], expw[:], rcol[:, 0:1], None,
                                            ALU.mult)
                    expt = spool.tile([128, 8 * 64], BF16, tag="expt")
                    for jt in range(8):
                        nc.sync.dma_start_transpose(
                            expt[:, jt * 64:(jt + 1) * 64],
                            expn[:, jt * 128:(jt + 1) * 128])
                    out_ps = opool.tile([D, 64], F32, tag="outp")
                    for jt in range(8):
                        nc.tensor.matmul(out_ps[:, :],
                                         r_sb[:, jt * D:(jt + 1) * D],
                                         expt[:, jt * 64:(jt + 1) * 64],
                                         start=(jt == 0), stop=(jt == 7),
                                         skip_group_check=True)
                    out_g.append(out_ps)
                ot_sb = otpool.tile([D, 128], F32, tag="ot")
                for g in range(2):
                    nc.scalar.copy(ot_sb[:, g * 64:(g + 1) * 64], out_g[g][:])
                nc.sync.dma_start(out=yout[rr, :, :], in_=ot_sb[:])

    _split_excess_waits(nc)
    return nc


_NC_CACHE = None


def _get_nc():
    global _NC_CACHE
    if _NC_CACHE is None:
        _NC_CACHE = _build_program()
    return _NC_CACHE


def kernel(x1, x2, r, Wk1, bk1, Wk2, bk2, Wq1, bq1, Wq2, bq2):
    global LAST_RESULT
    x1 = np.asarray(x1, np.float32)
    x2 = np.asarray(x2, np.float32)
    r = np.asarray(r, np.float32)
    Wk1 = np.asarray(Wk1, np.float32); bk1 = np.asarray(bk1, np.float32)
    Wk2 = np.asarray(Wk2, np.float32); bk2 = np.asarray(bk2, np.float32)
    Wq1 = np.asarray(Wq1, np.float32); bq1 = np.asarray(bq1, np.float32)
    Wq2 = np.asarray(Wq2, np.float32); bq2 = np.asarray(bq2, np.float32)

    # constant PE weights: ones-block lhsT, coefficient -2 (min-form pairs)
    # or +1 (abs-form pairs); column block s covers psum rows (2s, 2s+1).
    wones = np.zeros((128, 32 * 64), np.float32)
    cmask = np.zeros((1, 64), np.float32)
    for s in range(32):
        coef = 1.0 if _is_act_pair(s) else -2.0
        wones[0:64, s * 64 + 2 * s] = coef
        wones[64:128, s * 64 + 2 * s + 1] = coef
        if not _is_act_pair(s):
            cmask[0, 2 * s] = 1.0
            cmask[0, 2 * s + 1] = 1.0

    shared = {
        "wk1": Wk1, "bk1": bk1.reshape(D, 1),
        "wk2d": np.concatenate([Wk2, Wk2], axis=1),
        "bk2d": np.concatenate([bk2, bk2]).reshape(128, 1),
        "wq1": Wq1, "bq1": bq1.reshape(D, 1),
        "wq2": Wq2,
        "bq2d": np.concatenate([bq2, bq2]).reshape(128, 1),
        "wones": wones, "cmask": cmask,
        "ones64": np.ones((D, 1), np.float32),
    }
    shared = {k: np.ascontiguousarray(v) for k, v in shared.items()}

    in_maps = []
    for c in range(NCORES):
        b, h = c // 2, c % 2
        m = dict(shared)
        m["x1t"] = np.ascontiguousarray(x1[b].T)
        m["x2t"] = np.ascontiguousarray(x2[b, h * QSH:(h + 1) * QSH].T)
        m["rv"] = np.ascontiguousarray(r[b].astype(ml_dtypes.bfloat16))
        in_maps.append(m)

    nc = _get_nc()
    trace = bool(os.environ.get("BASS_TRACE"))
    if trace:
        _install_ntff_shim()
    res = bass_utils.run_bass_kernel_spmd(
        nc, in_maps, core_ids=list(range(NCORES)), trace=trace)
    LAST_RESULT = res

    # reassemble: yout[r, f, t] with t = g*64 + m, m = 2s + i2,
    # local query = i2*128 + r*64 + g*32 + s
    t = np.arange(128)
    g = t // 64
    m = t % 64
    s = m // 2
    i2 = m % 2
    out = np.empty((B, NQ, D), np.float32)
    for c in range(NCORES):
        b, h = c // 2, c % 2
        yc = res.results[c]["yout"]          # [2, D, 128]
        for rr in range(2):
            qloc = i2 * 128 + rr * 64 + g * 32 + s
            out[b, h * QSH + qloc, :] = yc[rr].T
    return out


# revision 6
# speedup vs baseline: 1.1999x; 1.1999x over previous
"""Laplace attention kernel for Trainium2, 8 NeuronCores.

Math (per batch b):
  k = MLP_k(x1[b])  [NK, D];  q = MLP_q(x2[b])  [NQ, D]
  dist[i,j] = sum_d |k[j,d] - q[i,d]|
  out = softmax_j(-dist) @ r[b]

Distribution: core c = (b, h) = (c//2, c%2): batch b, query-half h (256 queries).
Keys/values are replicated per batch pair of cores.

Per-core algorithm:
  - MLPs run transposed on the PE: kT2 [128=(i2,d), NK] holds kT stacked twice,
    q2T [128=(i2,d), 128] holds qT for queries (i2*128 + p).
  - For each query pair p, a [128, NK] tile M_p is produced:
      min-form pairs (DVE):  M_p = min(kT2, q_p)        (tensor_scalar, 2x fp32)
      abs-form pairs (ACT):  M_p = |kT2 - q_p|          (activation Abs, bias=q, scale=-1)
    Then one PE matmul per 512-column window reduces over the 128 partitions
    with a constant ones-block lhsT whose coefficient is -2 for min-form
    columns and +1 for abs-form columns, accumulating 32 pairs into one
    [64, 1024] PSUM tile. A K=1 correction matmul adds A_j = sum_d k[j,d]
    to min-form rows only (masked lhsT). The remaining B_i = sum_d q[i,d]
    offset is constant per row and cancels in softmax, and dist is large
    and positive, so exp(-P) needs no max-subtraction.
  - softmax: ACT Exp (scale=-1) with accum_out row-sum, DVE reciprocal +
    scale; weights stored bf16.
  - value matmul: DMA-transpose the bf16 weights to [j, q] layout, then PE
    per 128-key tile with r (bf16) as stationary operand, accumulating in
    PSUM; result is [D, queries], written out and transposed on the host.
"""

import os
import numpy as np
import ml_dtypes

import concourse.bass as bass
import concourse.mybir as mybir
from concourse.tile import TileContext
from concourse import bass_utils

B, NQ, NK, D = 4, 512, 1024, 64
NCORES = 8
QSH = NQ // 2           # queries per core
NPAIR = QSH // 2        # 128 query pairs per core
NWIN = NK // 512        # 512-column matmul windows
ACT_CNT_PER16 = 5       # s%16 < 5 -> abs-form pair on ACT engine

F32 = mybir.dt.float32
F32R = mybir.dt.float32r
BF16 = mybir.dt.bfloat16

LAST_RESULT = None      # BassKernelResults of the most recent run (for test.py)


def _is_act_pair(s):
    return (s % 16) < ACT_CNT_PER16


# ---------------------------------------------------------------------------
# walrus workaround: the CTRL-class instructions (Drain etc.) can carry only a
# few sem waits; hoist excess waits onto injected NoOps on the same engine.
def _split_excess_waits(nc, max_waits=1):
    for f in nc.m.functions:
        for bb in f.blocks:
            new_insts = []
            for inst in bb.instructions:
                si = inst.sync_info
                if si is not None and si.on_wait and len(si.on_wait) > max_waits:
                    waits = list(si.on_wait)
                    excess, keep = waits[:-max_waits], waits[-max_waits:]
                    for i in range(0, len(excess), max_waits):
                        nop = mybir.InstNoOp(
                            name=f"{inst.name}_waitsplit_{i // max_waits}",
                            ins=[], outs=[])
                        nop.engine = inst.engine
                        nop.sync_info = mybir.SyncInfo(
                            on_wait=excess[i:i + max_waits], on_update=[])
                        new_insts.append(nop)
                    si.on_wait = keep
                new_insts.append(inst)
            bb.instructions = new_insts


# shim antenv.axon_hooks (absent in this image) so BASS_TRACE=1 profiling works
def _install_ntff_shim():
    import sys, types
    if 'antenv.axon_hooks' in sys.modules:
        return
    try:
        mod = types.ModuleType('antenv.axon_hooks')
        state = {}
        mod.set_axon_ntff_profile_hook = lambda h: state.__setitem__('h', h)
        mod.get_axon_ntff_profile_hook = lambda: state.get('h')
        sys.modules['antenv.axon_hooks'] = mod
        import antenv
        antenv.axon_hooks = mod
        from trn_agent_boot.trn_boot import _ntff_profile_via_ctypes
        h = _ntff_profile_via_ctypes('/opt/axon/libaxon_pjrt.so')
        if h is not None:
            mod.set_axon_ntff_profile_hook(h)
    except Exception:
        pass


# ---------------------------------------------------------------------------
def _build_program():
    nc = bass.Bass("TRN2")

    x1t = nc.dram_tensor("x1t", [D, NK], F32, kind="ExternalInput")
    x2t = nc.dram_tensor("x2t", [D, QSH], F32, kind="ExternalInput")
    rv = nc.dram_tensor("rv", [NK, D], BF16, kind="ExternalInput")
    wk1 = nc.dram_tensor("wk1", [D, D], F32, kind="ExternalInput")
    bk1 = nc.dram_tensor("bk1", [D, 1], F32, kind="ExternalInput")
    wk2d = nc.dram_tensor("wk2d", [D, 128], F32, kind="ExternalInput")
    bk2d = nc.dram_tensor("bk2d", [128, 1], F32, kind="ExternalInput")
    wq1 = nc.dram_tensor("wq1", [D, D], F32, kind="ExternalInput")
    bq1 = nc.dram_tensor("bq1", [D, 1], F32, kind="ExternalInput")
    wq2 = nc.dram_tensor("wq2", [D, D], F32, kind="ExternalInput")
    bq2d = nc.dram_tensor("bq2d", [128, 1], F32, kind="ExternalInput")
    wones = nc.dram_tensor("wones", [128, 32 * 64], F32R, kind="ExternalInput")
    cmask = nc.dram_tensor("cmask", [1, 64], F32R, kind="ExternalInput")
    ones64 = nc.dram_tensor("ones64", [D, 1], F32, kind="ExternalInput")
    yout = nc.dram_tensor("yout", [2, D, 128], F32, kind="ExternalOutput")

    ACT = mybir.ActivationFunctionType
    ALU = mybir.AluOpType

    with TileContext(nc) as tc:
        import contextlib
        with contextlib.ExitStack() as ctx:
            consts = ctx.enter_context(tc.tile_pool(name="consts", bufs=1))

            x1t_sb = consts.tile([D, NK], F32)
            x2t_sb = consts.tile([D, QSH], F32)
            r_sb = consts.tile([128, 8 * D], BF16)
            wk1_sb = consts.tile([D, D], F32)
            bk1_sb = consts.tile([D, 1], F32)
            wk2d_sb = consts.tile([D, 128], F32)
            bk2d_sb = consts.tile([128, 1], F32)
            wq1_sb = consts.tile([D, D], F32)
            bq1_sb = consts.tile([D, 1], F32)
            wq2_sb = consts.tile([D, D], F32)
            bq2d_sb = consts.tile([128, 1], F32)
            wones_sb = consts.tile([128, 32 * 64], F32R)
            cmask_sb = consts.tile([1, 64], F32R)
            ones64_sb = consts.tile([D, 1], F32)

            nc.sync.dma_start(out=x1t_sb[:], in_=x1t[:, :])
            nc.sync.dma_start(out=x2t_sb[:], in_=x2t[:, :])
            for jt in range(8):
                nc.sync.dma_start(out=r_sb[:, jt * D:(jt + 1) * D],
                                  in_=rv[jt * 128:(jt + 1) * 128, :])
            nc.sync.dma_start(out=wk1_sb[:], in_=wk1[:, :])
            nc.sync.dma_start(out=bk1_sb[:], in_=bk1[:, :])
            nc.sync.dma_start(out=wk2d_sb[:], in_=wk2d[:, :])
            nc.sync.dma_start(out=bk2d_sb[:], in_=bk2d[:, :])
            nc.sync.dma_start(out=wq1_sb[:], in_=wq1[:, :])
            nc.sync.dma_start(out=bq1_sb[:], in_=bq1[:, :])
            nc.sync.dma_start(out=wq2_sb[:], in_=wq2[:, :])
            nc.sync.dma_start(out=bq2d_sb[:], in_=bq2d[:, :])
            nc.sync.dma_start(out=wones_sb[:], in_=wones[:, :])
            nc.sync.dma_start(out=cmask_sb[:], in_=cmask[:, :])
            nc.sync.dma_start(out=ones64_sb[:], in_=ones64[:, :])

            kt2_sb = consts.tile([128, NK], F32)
            q2t_sb = consts.tile([128, 128], F32)
            ht_sb = consts.tile([D, NK], F32)
            hqt_sb = consts.tile([D, QSH], F32)
            arow_sb = consts.tile([1, NK], F32R)

            # ---- MLPs (transposed) ----
            with tc.tile_pool(name="mlppsum", bufs=2, space="PSUM") as mp:
                for w in range(NWIN):
                    ph = mp.tile([D, 512], F32, tag="ph")
                    nc.tensor.matmul(ph[:], wk1_sb[:], x1t_sb[:, w * 512:(w + 1) * 512],
                                     start=True, stop=True)
                    nc.scalar.activation(ht_sb[:, w * 512:(w + 1) * 512], ph[:],
                                         ACT.Relu, bias=bk1_sb[:, 0:1], scale=1.0)
                for w in range(NWIN):
                    pk = mp.tile([128, 512], F32, tag="pk")
                    nc.tensor.matmul(pk[:], wk2d_sb[:], ht_sb[:, w * 512:(w + 1) * 512],
                                     start=True, stop=True)
                    nc.scalar.activation(kt2_sb[:, w * 512:(w + 1) * 512], pk[:],
                                         ACT.Identity, bias=bk2d_sb[:, 0:1], scale=1.0)
                phq = mp.tile([D, QSH], F32, tag="ph")
                nc.tensor.matmul(phq[:], wq1_sb[:], x2t_sb[:], start=True, stop=True)
                nc.scalar.activation(hqt_sb[:], phq[:], ACT.Relu,
                                     bias=bq1_sb[:, 0:1], scale=1.0)
                pq = mp.tile([128, 128], F32, tag="pk")
                nc.tensor.matmul(pq[0:64, :], wq2_sb[:], hqt_sb[:, 0:128],
                                 start=True, stop=False, skip_group_check=True)
                nc.tensor.matmul(pq[64:128, :], wq2_sb[:], hqt_sb[:, 128:256],
                                 start=True, stop=True, skip_group_check=True)
                nc.scalar.activation(q2t_sb[:], pq[:], ACT.Identity,
                                     bias=bq2d_sb[:, 0:1], scale=1.0)
                # A_j = sum_d k[j, d]  (fp32 exact), stored f32r for corrections
                pa = mp.tile([1, NK], F32, tag="pa")
                for w in range(NWIN):
                    nc.tensor.matmul(pa[:, w * 512:(w + 1) * 512], ones64_sb[:],
                                     kt2_sb[0:64, w * 512:(w + 1) * 512],
                                     start=True, stop=True, skip_group_check=True)
                nc.vector.tensor_copy(arow_sb[:], pa[:])

            # ---- main loop ----
            mpool = ctx.enter_context(tc.tile_pool(name="mtiles", bufs=3))
            dpool = ctx.enter_context(
                tc.tile_pool(name="dist", bufs=3, space="PSUM"))
            opool = ctx.enter_context(
                tc.tile_pool(name="outp", bufs=2, space="PSUM"))
            spool = ctx.enter_context(tc.tile_pool(name="smax", bufs=3))
            otpool = ctx.enter_context(tc.tile_pool(name="outs", bufs=2))

            ot_sbs = {}

            def make_tail(rr, dists):
                def tail():
                    # softmax over keys (no shift needed: dist >> 0)
                    expw = spool.tile([128, NK], BF16, tag="expw")
                    ssum = spool.tile([128, 1], F32, tag="ssum")
                    expw1 = spool.tile([64, NK], BF16, tag="expw1")
                    ssum1 = spool.tile([64, 1], F32, tag="ssum1")
                    rcol = spool.tile([128, 1], F32, tag="rcol")
                    nc.scalar.activation(expw[0:64, :], dists[0][:], ACT.Exp,
                                         bias=0.0, scale=-1.0,
                                         accum_out=ssum[0:64, 0:1])
                    nc.scalar.activation(expw1[:], dists[1][:], ACT.Exp,
                                         bias=0.0, scale=-1.0,
                                         accum_out=ssum1[:, 0:1])
                    # engines cannot shift partitions; SBUF->SBUF DMA can
                    nc.sync.dma_start(out=expw[64:128, :], in_=expw1[:])
                    nc.scalar.dma_start(out=ssum[64:128, :], in_=ssum1[:])
                    nc.vector.reciprocal(rcol[:], ssum[:])
                    expn = spool.tile([128, NK], BF16, tag="expn")
                    nc.vector.tensor_scalar(expn[:], expw[:], rcol[:, 0:1], None,
                                            ALU.mult)
                    expt = spool.tile([128, 8 * 128], BF16, tag="expt")
                    for jt in range(8):
                        eng = nc.sync if jt % 2 == 0 else nc.scalar
                        eng.dma_start_transpose(
                            expt[:, jt * 128:(jt + 1) * 128],
                            expn[:, jt * 128:(jt + 1) * 128])
                    out_ps = opool.tile([D, 128], F32, tag="outp")
                    for jt in range(8):
                        nc.tensor.matmul(out_ps[:, :],
                                         r_sb[:, jt * D:(jt + 1) * D],
                                         expt[:, jt * 128:(jt + 1) * 128],
                                         start=(jt == 0), stop=(jt == 7),
                                         skip_group_check=True)
                    ot = ot_sbs[rr]
                    nc.scalar.copy(ot[:], out_ps[:])
                    nc.sync.dma_start(out=yout[rr, :, :], in_=ot[:])
                return tail

            pending_tail = None
            for rr in range(2):
                ot_sbs[rr] = otpool.tile([D, 128], F32, name="ot", tag="ot")
                dists = []
                for g in range(2):
                    dist = dpool.tile([64, NK], F32, name="dist", tag="dist")
                    dists.append(dist)
                    for s in range(32):
                        p = rr * 64 + g * 32 + s
                        mt = mpool.tile([128, NK], F32R, tag="mt")
                        if _is_act_pair(s):
                            nc.scalar.activation(mt[:], kt2_sb[:], ACT.Abs,
                                                 bias=q2t_sb[:, p:p + 1], scale=-1.0)
                        else:
                            nc.vector.tensor_scalar(mt[:], kt2_sb[:],
                                                    q2t_sb[:, p:p + 1], None, ALU.min)
                        for w in range(NWIN):
                            nc.tensor.matmul(
                                dist[:, w * 512:(w + 1) * 512],
                                wones_sb[:, s * 64:(s + 1) * 64],
                                mt[:, w * 512:(w + 1) * 512],
                                start=(s == 0), stop=False, skip_group_check=True)
                        if g == 0 and s == 16 and pending_tail is not None:
                            pending_tail()
                            pending_tail = None
                    # A_j correction on min-form rows only (masked K=1 matmul)
                    for w in range(NWIN):
                        nc.tensor.matmul(
                            dist[:, w * 512:(w + 1) * 512],
                            cmask_sb[:],
                            arow_sb[:, w * 512:(w + 1) * 512],
                            start=False, stop=True, skip_group_check=True)
                pending_tail = make_tail(rr, dists)
            pending_tail()

    _split_excess_waits(nc)
    return nc


_NC_CACHE = None


def _get_nc():
    global _NC_CACHE
    if _NC_CACHE is None:
        _NC_CACHE = _build_program()
    return _NC_CACHE


def kernel(x1, x2, r, Wk1, bk1, Wk2, bk2, Wq1, bq1, Wq2, bq2):
    global LAST_RESULT
    x1 = np.asarray(x1, np.float32)
    x2 = np.asarray(x2, np.float32)
    r = np.asarray(r, np.float32)
    Wk1 = np.asarray(Wk1, np.float32); bk1 = np.asarray(bk1, np.float32)
    Wk2 = np.asarray(Wk2, np.float32); bk2 = np.asarray(bk2, np.float32)
    Wq1 = np.asarray(Wq1, np.float32); bq1 = np.asarray(bq1, np.float32)
    Wq2 = np.asarray(Wq2, np.float32); bq2 = np.asarray(bq2, np.float32)

    # constant PE weights: ones-block lhsT, coefficient -2 (min-form pairs)
    # or +1 (abs-form pairs); column block s covers psum rows (2s, 2s+1).
    wones = np.zeros((128, 32 * 64), np.float32)
    cmask = np.zeros((1, 64), np.float32)
    for s in range(32):
        coef = 1.0 if _is_act_pair(s) else -2.0
        wones[0:64, s * 64 + 2 * s] = coef
        wones[64:128, s * 64 + 2 * s + 1] = coef
        if not _is_act_pair(s):
            cmask[0, 2 * s] = 1.0
            cmask[0, 2 * s + 1] = 1.0

    shared = {
        "wk1": Wk1, "bk1": bk1.reshape(D, 1),
        "wk2d": np.concatenate([Wk2, Wk2], axis=1),
        "bk2d": np.concatenate([bk2, bk2]).reshape(128, 1),
        "wq1": Wq1, "bq1": bq1.reshape(D, 1),
        "wq2": Wq2,
        "bq2d": np.concatenate([bq2, bq2]).reshape(128, 1),
        "wones": wones, "cmask": cmask,
        "ones64": np.ones((D, 1), np.float32),
    }
    shared = {k: np.ascontiguousarray(v) for k, v in shared.items()}

    in_maps = []
    for c in range(NCORES):
        b, h = c // 2, c % 2
        m = dict(shared)
        m["x1t"] = np.ascontiguousarray(x1[b].T)
        m["x2t"] = np.ascontiguousarray(x2[b, h * QSH:(h + 1) * QSH].T)
        m["rv"] = np.ascontiguousarray(r[b].astype(ml_dtypes.bfloat16))
        in_maps.append(m)

    nc = _get_nc()
    trace = bool(os.environ.get("BASS_TRACE"))
    if trace:
        _install_ntff_shim()
    res = bass_utils.run_bass_kernel_spmd(
        nc, in_maps, core_ids=list(range(NCORES)), trace=trace)
    LAST_RESULT = res

    # reassemble: yout[r, f, t] with t = g*64 + m, m = 2s + i2,
    # local query = i2*128 + r*64 + g*32 + s
    t = np.arange(128)
    g = t // 64
    m = t % 64
    s = m // 2
    i2 = m % 2
    out = np.empty((B, NQ, D), np.float32)
    for c in range(NCORES):
        b, h = c // 2, c % 2
        yc = res.results[c]["yout"]          # [2, D, 128]
        for rr in range(2):
            qloc = i2 * 128 + rr * 64 + g * 32 + s
            out[b, h * QSH + qloc, :] = yc[rr].T
    return out


# revision 50
# speedup vs baseline: 1.8214x; 1.5179x over previous
"""Laplace attention kernel for Trainium2, 8 NeuronCores.

Math (per batch b):
  k = MLP_k(x1[b])  [NK, D];  q = MLP_q(x2[b])  [NQ, D]
  dist[i,j] = sum_d |k[j,d] - q[i,d]|
  out = softmax_j(-dist) @ r[b]

Distribution: core c = (b, h) = (c//2, c%2): batch b, query-half h (256 queries).
Keys/values are replicated per batch pair of cores.

Per-core algorithm:
  - MLPs run transposed on the PE: kT2 [128=(i2,d), NK] holds kT stacked twice,
    q2T [128=(i2,d), 128] holds qT for queries (i2*128 + p).
  - For each query pair p, a [128, NK] tile M_p is produced:
      min-form pairs (DVE):  M_p = min(kT2, q_p)        (tensor_scalar, 2x fp32)
      abs-form pairs (ACT):  M_p = |kT2 - q_p|          (activation Abs, bias=q, scale=-1)
    Then one PE matmul per 512-column window reduces over the 128 partitions
    with a constant ones-block lhsT whose coefficient is -2 for min-form
    columns and +1 for abs-form columns, accumulating 32 pairs into one
    [64, 1024] PSUM tile. A K=1 correction matmul adds A_j = sum_d k[j,d]
    to min-form rows only (masked lhsT). The remaining B_i = sum_d q[i,d]
    offset is constant per row and cancels in softmax, and dist is large
    and positive, so exp(-P) needs no max-subtraction.
  - softmax: ACT Exp (scale=-1) with accum_out row-sum, DVE reciprocal +
    scale; weights stored bf16.
  - value matmul: DMA-transpose the bf16 weights to [j, q] layout, then PE
    per 128-key tile with r (bf16) as stationary operand, accumulating in
    PSUM; result is [D, queries], written out and transposed on the host.
"""

import os
import numpy as np
import ml_dtypes

import concourse.bass as bass
import concourse.mybir as mybir
from concourse.tile import TileContext
from concourse import bass_utils

B, NQ, NK, D = 4, 512, 1024, 64
NCORES = 8
QSH = NQ // 2           # queries per core
NPAIR = QSH // 2        # 128 query pairs per core
NWIN = NK // 512        # 512-column matmul windows
ACT_SLOTS = (1, 4, 8, 11, 15, 18, 22, 25, 29)  # slots within later groups

F32 = mybir.dt.float32
F32R = mybir.dt.float32r
F16 = mybir.dt.float16
BF16 = mybir.dt.bfloat16

LAST_RESULT = None      # BassKernelResults of the most recent run (for test.py)



def _is_act_pair(p):
    # no ACT pairs among the first few: their Mt tiles must fill the pipeline
    # while the ACT engine is still busy with the MLP chain
    return p >= 8 and (p % 32) in ACT_SLOTS


# ---------------------------------------------------------------------------
# walrus workaround: the CTRL-class instructions (Drain etc.) can carry only a
# few sem waits; hoist excess waits onto injected NoOps on the same engine.
def _split_excess_waits(nc, max_waits=1):
    for f in nc.m.functions:
        for bb in f.blocks:
            new_insts = []
            for inst in bb.instructions:
                si = inst.sync_info
                if si is not None and si.on_wait and len(si.on_wait) > max_waits:
                    waits = list(si.on_wait)
                    excess, keep = waits[:-max_waits], waits[-max_waits:]
                    for i in range(0, len(excess), max_waits):
                        nop = mybir.InstNoOp(
                            name=f"{inst.name}_waitsplit_{i // max_waits}",
                            ins=[], outs=[])
                        nop.engine = inst.engine
                        nop.sync_info = mybir.SyncInfo(
                            on_wait=excess[i:i + max_waits], on_update=[])
                        new_insts.append(nop)
                    si.on_wait = keep
                new_insts.append(inst)
            bb.instructions = new_insts


# shim antenv.axon_hooks (absent in this image) so BASS_TRACE=1 profiling works
def _install_ntff_shim():
    import sys, types
    if 'antenv.axon_hooks' in sys.modules:
        return
    try:
        mod = types.ModuleType('antenv.axon_hooks')
        state = {}
        mod.set_axon_ntff_profile_hook = lambda h: state.__setitem__('h', h)
        mod.get_axon_ntff_profile_hook = lambda: state.get('h')
        sys.modules['antenv.axon_hooks'] = mod
        import antenv
        antenv.axon_hooks = mod
        from trn_agent_boot.trn_boot import _ntff_profile_via_ctypes
        h = _ntff_profile_via_ctypes('/opt/axon/libaxon_pjrt.so')
        if h is not None:
            mod.set_axon_ntff_profile_hook(h)
    except Exception:
        pass


# ---------------------------------------------------------------------------
def _build_program():
    nc = bass.Bass("TRN2")

    x1t = nc.dram_tensor("x1t", [D, NK], F16, kind="ExternalInput")
    x2t = nc.dram_tensor("x2t", [D, QSH], F16, kind="ExternalInput")
    rv = nc.dram_tensor("rv", [NK, D], BF16, kind="ExternalInput")
    wk1 = nc.dram_tensor("wk1", [D, D], F16, kind="ExternalInput")
    bk1 = nc.dram_tensor("bk1", [D, 1], F32, kind="ExternalInput")
    wk2d = nc.dram_tensor("wk2d", [D, 128], F16, kind="ExternalInput")
    bk2d = nc.dram_tensor("bk2d", [128, 1], F32, kind="ExternalInput")
    wq1 = nc.dram_tensor("wq1", [D, D], F16, kind="ExternalInput")
    bq1 = nc.dram_tensor("bq1", [D, 1], F32, kind="ExternalInput")
    wq2 = nc.dram_tensor("wq2", [D, D], F16, kind="ExternalInput")
    bq2d = nc.dram_tensor("bq2d", [128, 1], F32, kind="ExternalInput")
    wones = nc.dram_tensor("wones", [128, 64 * 64], F16, kind="ExternalInput")
    cmask = nc.dram_tensor("cmask", [1, 192], F16, kind="ExternalInput")
    ones64 = nc.dram_tensor("ones64", [D, 1], F16, kind="ExternalInput")
    ident = nc.dram_tensor("ident", [128, 128], BF16, kind="ExternalInput")
    yout = nc.dram_tensor("yout", [2, D, 128], F32, kind="ExternalOutput")
    sout = nc.dram_tensor("sout", [2, 128], F32, kind="ExternalOutput")

    ACT = mybir.ActivationFunctionType
    ALU = mybir.AluOpType

    with TileContext(nc) as tc:
        import contextlib
        with contextlib.ExitStack() as ctx:
            consts = ctx.enter_context(tc.tile_pool(name="consts", bufs=1))

            x1t_sb = consts.tile([D, NK], F16)
            x2t_sb = consts.tile([D, QSH], F16)
            r_sb = consts.tile([128, 8 * D], BF16)
            wk1_sb = consts.tile([D, D], F16)
            bk1_sb = consts.tile([D, 1], F32)
            wk2d_sb = consts.tile([D, 128], F16)
            bk2d_sb = consts.tile([128, 1], F32)
            wq1_sb = consts.tile([D, D], F16)
            bq1_sb = consts.tile([D, 1], F32)
            wq2_sb = consts.tile([D, D], F16)
            bq2d_sb = consts.tile([128, 1], F32)
            wones_sb = consts.tile([128, 64 * 64], F16)
            cmask_sb = consts.tile([1, 192], F16)
            ones64_sb = consts.tile([D, 1], F16)
            ident_sb = consts.tile([128, 128], BF16)

            nc.scalar.dma_start(out=wq1_sb[:], in_=wq1[:, :])
            nc.scalar.dma_start(out=bq1_sb[:], in_=bq1[:, :])
            nc.scalar.dma_start(out=x2t_sb[:], in_=x2t[:, :])
            nc.scalar.dma_start(out=wq2_sb[:], in_=wq2[:, :])
            nc.scalar.dma_start(out=bq2d_sb[:], in_=bq2d[:, :])
            nc.sync.dma_start(out=x1t_sb[:, 0:512], in_=x1t[:, 0:512])
            nc.scalar.dma_start(out=x1t_sb[:, 512:1024], in_=x1t[:, 512:1024])
            nc.sync.dma_start(out=wk1_sb[:], in_=wk1[:, :])
            nc.sync.dma_start(out=bk1_sb[:], in_=bk1[:, :])
            nc.sync.dma_start(out=wk2d_sb[:], in_=wk2d[:, :])
            nc.sync.dma_start(out=bk2d_sb[:], in_=bk2d[:, :])
            nc.gpsimd.dma_start(out=wones_sb[:], in_=wones[:, :])
            nc.gpsimd.dma_start(out=cmask_sb[:], in_=cmask[:, :])
            nc.gpsimd.dma_start(out=ones64_sb[:], in_=ones64[:, :])
            nc.gpsimd.dma_start(out=ident_sb[:], in_=ident[:, :])
            for jt in range(8):
                nc.gpsimd.dma_start(out=r_sb[:, jt * D:(jt + 1) * D],
                                    in_=rv[jt * 128:(jt + 1) * 128, :])

            kt2_sb = consts.tile([128, NK], F16)
            q2t_sb = consts.tile([128, 128], F32)
            ht_sb = consts.tile([D, NK], F16)
            hqt_sb = consts.tile([D, QSH], F16)
            arow_sb = consts.tile([1, NK], F16)

            # ---- MLPs (transposed) ----
            # q-path first: q2t gates every Mt producer. All evacuations on
            # ACT so the DVE can start min production immediately after.
            with tc.tile_pool(name="mlppsum", bufs=2, space="PSUM") as mp:
                phq = mp.tile([D, QSH], F32, tag="ph")
                nc.tensor.matmul(phq[:], wq1_sb[:], x2t_sb[:], start=True, stop=True)
                nc.scalar.activation(hqt_sb[:], phq[:], ACT.Relu,
                                     bias=bq1_sb[:, 0:1], scale=1.0)
                pq = mp.tile([128, 128], F32, tag="pk")
                nc.tensor.matmul(pq[0:64, :], wq2_sb[:], hqt_sb[:, 0:128],
                                 start=True, stop=False, skip_group_check=True)
                nc.tensor.matmul(pq[64:128, :], wq2_sb[:], hqt_sb[:, 128:256],
                                 start=True, stop=True, skip_group_check=True)
                nc.scalar.activation(q2t_sb[:], pq[:], ACT.Identity,
                                     bias=bq2d_sb[:, 0:1], scale=1.0)
                for w in range(NWIN):
                    ph = mp.tile([D, 512], F32, tag="ph")
                    nc.tensor.matmul(ph[:], wk1_sb[:], x1t_sb[:, w * 512:(w + 1) * 512],
                                     start=True, stop=True)
                    nc.scalar.activation(ht_sb[:, w * 512:(w + 1) * 512], ph[:],
                                         ACT.Relu, bias=bk1_sb[:, 0:1], scale=1.0)
                    pk = mp.tile([128, 512], F32, tag="pk")
                    nc.tensor.matmul(pk[:], wk2d_sb[:], ht_sb[:, w * 512:(w + 1) * 512],
                                     start=True, stop=True)
                    nc.scalar.activation(kt2_sb[:, w * 512:(w + 1) * 512], pk[:],
                                         ACT.Identity, bias=bk2d_sb[:, 0:1], scale=1.0)
                # A_j = sum_d k[j, d] (same fp16 k the min path sees)
                pa = mp.tile([1, NK], F32, tag="pa")
                for w in range(NWIN):
                    nc.tensor.matmul(pa[:, w * 512:(w + 1) * 512], ones64_sb[:],
                                     kt2_sb[0:64, w * 512:(w + 1) * 512],
                                     start=True, stop=True, skip_group_check=True)
                nc.scalar.copy(arow_sb[:], pa[:])

            # ---- main loop ----
            mpool = ctx.enter_context(tc.tile_pool(name="mtiles", bufs=6))
            dpool = ctx.enter_context(
                tc.tile_pool(name="dist", bufs=3, space="PSUM"))
            opool = ctx.enter_context(
                tc.tile_pool(name="outp", bufs=2, space="PSUM"))
            spool = ctx.enter_context(tc.tile_pool(name="smax", bufs=3))
            otpool = ctx.enter_context(tc.tile_pool(name="outs", bufs=2))

            ot_sbs = {}

            def make_tail(rr, dists, final=False):
                state = {}

                def exp0():
                    expw = spool.tile([128, NK], BF16, tag="expw")
                    ssum = spool.tile([128, 1], F32, tag="ssum")
                    state["expw"], state["ssum"] = expw, ssum
                    nc.scalar.activation(expw[0:64, :], dists[0][:], ACT.Exp,
                                         bias=0.0, scale=-1.0,
                                         accum_out=ssum[0:64, 0:1])

                def exp1_tp0():
                    expw = state["expw"]
                    expw1 = spool.tile([64, NK], BF16, tag="expw1")
                    ssum1 = spool.tile([64, 1], F32, tag="ssum1")
                    state["expw1"], state["ssum1"] = expw1, ssum1
                    nc.scalar.activation(expw1[:], dists[1][:], ACT.Exp,
                                         bias=0.0, scale=-1.0,
                                         accum_out=ssum1[:, 0:1])
                    # transpose on the tensor engine (psum) + DVE copies:
                    # cheaper and conflict-free vs the xbar DMA path
                    expt = spool.tile([128, 8 * 128], BF16, tag="expt")
                    state["expt"] = expt
                    for jt in range(8):
                        tp = opool.tile([128, 64], BF16, tag="outp")
                        nc.tensor.transpose(tp[:], expw[0:64, jt * 128:(jt + 1) * 128],
                                            ident_sb[0:64, 0:64])
                        nc.vector.tensor_copy(
                            expt[:, jt * 128:jt * 128 + 64], tp[:])

                def tp1():
                    expw1, ssum, ssum1 = state["expw1"], state["ssum"], state["ssum1"]
                    expt = state["expt"]
                    nc.gpsimd.dma_start(out=sout[rr, 0:64], in_=ssum[0:64, 0])
                    nc.gpsimd.dma_start(out=sout[rr, 64:128], in_=ssum1[:, 0])
                    for jt in range(8):
                        tp = opool.tile([128, 64], BF16, tag="outp")
                        nc.tensor.transpose(tp[:], expw1[:, jt * 128:(jt + 1) * 128],
                                            ident_sb[0:64, 0:64])
                        nc.vector.tensor_copy(
                            expt[:, jt * 128 + 64:jt * 128 + 128], tp[:])

                def value():
                    expt = state["expt"]
                    out_ps = opool.tile([D, 128], F32, tag="outp")
                    for jt in range(8):
                        nc.tensor.matmul(out_ps[:, :],
                                         r_sb[:, jt * D:(jt + 1) * D],
                                         expt[:, jt * 128:(jt + 1) * 128],
                                         start=(jt == 0), stop=(jt == 7),
                                         skip_group_check=True)
                    ot = ot_sbs[rr]
                    nc.scalar.copy(ot[:], out_ps[:])
                    nc.sync.dma_start(out=yout[rr, :, :], in_=ot[:])
                return exp0, exp1_tp0, tp1, value

            prev = None
            for rr in range(2):
                ot_sbs[rr] = otpool.tile([D, 128], F32, name="ot", tag="ot")
                dists = []
                for g in range(2):
                    dist = dpool.tile([64, NK], F32, name="dist", tag="dist")
                    dists.append(dist)
                    for s in range(32):
                        p = rr * 64 + g * 32 + s
                        mt = mpool.tile([128, NK], F16, tag="mt")
                        if _is_act_pair(p):
                            nc.scalar.activation(mt[:], kt2_sb[:], ACT.Abs,
                                                 bias=q2t_sb[:, p:p + 1], scale=-1.0)
                            bi = s + 32
                        elif p < 8:
                            # per-window halves: lets window-0 matmuls start
                            # before the second kt2 window is computed
                            for w in range(NWIN):
                                nc.vector.tensor_scalar(
                                    mt[:, w * 512:(w + 1) * 512],
                                    kt2_sb[:, w * 512:(w + 1) * 512],
                                    q2t_sb[:, p:p + 1], None, ALU.min)
                            bi = s
                        else:
                            nc.vector.tensor_scalar(mt[:], kt2_sb[:],
                                                    q2t_sb[:, p:p + 1], None, ALU.min)
                            bi = s
                        for w in range(NWIN):
                            nc.tensor.matmul(
                                dist[:, w * 512:(w + 1) * 512],
                                wones_sb[:, bi * 64:(bi + 1) * 64],
                                mt[:, w * 512:(w + 1) * 512],
                                start=(s == 0), stop=False, skip_group_check=True)
                        if g == 0 and prev is not None:
                            if s == 4:
                                prev[2]()      # g1-half transposes of prev round
                            elif s == 16:
                                prev[3]()      # value matmuls of prev round
                                prev = None
                    cm0 = 0 if (rr == 0 and g == 0) else 64
                    for w in range(NWIN):
                        nc.tensor.matmul(
                            dist[:, w * 512:(w + 1) * 512],
                            cmask_sb[:, cm0:cm0 + 64],
                            arow_sb[:, w * 512:(w + 1) * 512],
                            start=False, stop=True, skip_group_check=True)
                    cur = make_tail(rr, dists, final=(rr == 1)) if g == 0 else cur
                    if g == 0:
                        cur[0]()               # exp of g0
                    else:
                        cur[1]()               # exp g1 + g0-half transposes
                        prev = cur
            prev[2]()
            prev[3]()

    _split_excess_waits(nc)
    return nc


_NC_CACHE = None


def _get_nc():
    global _NC_CACHE
    if _NC_CACHE is None:
        _NC_CACHE = _build_program()
    return _NC_CACHE


def kernel(x1, x2, r, Wk1, bk1, Wk2, bk2, Wq1, bq1, Wq2, bq2):
    global LAST_RESULT
    x1 = np.asarray(x1, np.float32)
    x2 = np.asarray(x2, np.float32)
    r = np.asarray(r, np.float32)
    Wk1 = np.asarray(Wk1, np.float32); bk1 = np.asarray(bk1, np.float32)
    Wk2 = np.asarray(Wk2, np.float32); bk2 = np.asarray(bk2, np.float32)
    Wq1 = np.asarray(Wq1, np.float32); bq1 = np.asarray(bq1, np.float32)
    Wq2 = np.asarray(Wq2, np.float32); bq2 = np.asarray(bq2, np.float32)

    # constant PE weights: ones-block lhsT; blocks 0-31 carry coefficient -2
    # (min-form), blocks 32-63 carry +1 (abs-form). Column block s covers psum
    # rows (2s, 2s+1). cmask: A_j-correction masks (group0 | later groups).
    wones = np.zeros((128, 64 * 64), np.float32)
    cmask = np.zeros((1, 192), np.float32)
    for s in range(32):
        wones[0:64, s * 64 + 2 * s] = -2.0
        wones[64:128, s * 64 + 2 * s + 1] = -2.0
        wones[0:64, (s + 32) * 64 + 2 * s] = 1.0
        wones[64:128, (s + 32) * 64 + 2 * s + 1] = 1.0
        if not _is_act_pair(s):            # group 0 (p = s)
            cmask[0, 2 * s] = 1.0
            cmask[0, 2 * s + 1] = 1.0
        if not _is_act_pair(32 + s):       # groups 1-3
            cmask[0, 64 + 2 * s] = 1.0
            cmask[0, 64 + 2 * s + 1] = 1.0
    shared = {
        "wk1": Wk1.astype(np.float16), "bk1": bk1.reshape(D, 1),
        "wk2d": np.concatenate([Wk2, Wk2], axis=1).astype(np.float16),
        "bk2d": np.concatenate([bk2, bk2]).reshape(128, 1),
        "wq1": Wq1.astype(np.float16), "bq1": bq1.reshape(D, 1),
        "wq2": Wq2.astype(np.float16),
        "bq2d": np.concatenate([bq2, bq2]).reshape(128, 1),
        "wones": wones.astype(np.float16), "cmask": cmask.astype(np.float16),
        "ones64": np.ones((D, 1), np.float16),
        "ident": np.eye(128, dtype=ml_dtypes.bfloat16),
    }
    shared = {k: np.ascontiguousarray(v) for k, v in shared.items()}

    in_maps = []
    for c in range(NCORES):
        b, h = c // 2, c % 2
        m = dict(shared)
        m["x1t"] = np.ascontiguousarray(x1[b].T.astype(np.float16))
        m["x2t"] = np.ascontiguousarray(x2[b, h * QSH:(h + 1) * QSH].T.astype(np.float16))
        m["rv"] = np.ascontiguousarray(r[b].astype(ml_dtypes.bfloat16))
        in_maps.append(m)

    nc = _get_nc()
    trace = bool(os.environ.get("BASS_TRACE"))
    if trace:
        _install_ntff_shim()
    res = None
    for attempt in range(3):
        try:
            res = bass_utils.run_bass_kernel_spmd(
                nc, in_maps, core_ids=list(range(NCORES)), trace=trace)
            break
        except Exception:
            # transient NRT_EXEC_UNIT_UNRECOVERABLE failures have been
            # observed on this fabric; retry (compile results are cached)
            if attempt == 2:
                raise
            import time
            time.sleep(5)
    LAST_RESULT = res

    # reassemble: yout[r, f, t] with t = g*64 + m, m = 2s + i2,
    # local query = i2*128 + r*64 + g*32 + s
    t = np.arange(128)
    g = t // 64
    m = t % 64
    s = m // 2
    i2 = m % 2
    out = np.empty((B, NQ, D), np.float32)
    for c in range(NCORES):
        b, h = c // 2, c % 2
        yc = res.results[c]["yout"]          # [2, D, 128]
        sc = res.results[c]["sout"]          # [2, 128]
        for rr in range(2):
            qloc = i2 * 128 + rr * 64 + g * 32 + s
            out[b, h * QSH + qloc, :] = (yc[rr] / sc[rr][None, :]).T
    return out
